# revision 15
# baseline (speedup 1.0000x reference)
"""VQ codebook kernel for Trainium2 (8 NeuronCores, data-parallel over tokens).

Problem: x [32768, 256] f32, codebook [8192, 256] f32.
  encoding = argmin_k ||x - c_k||^2          (int32)
  nearest  = codebook[encoding]              (f32)
  codebook_loss = encoder_loss = mean((x - nearest)^2)

Device strategy (per core, 4096 tokens):
  - score[t, k] = x_t . c_k - 0.5*||c_k||^2 ; argmin dist == argmax score.
  - PE: fp32 matmuls xT-tile [128d x 128t] x cbT [128d x Kslice] into PSUM,
    accumulated over the two 128-row halves of D=256.
  - DVE tensor_tensor_reduce: adds the -0.5*||c||^2 bias (precomputed on
    device via an all-ones matmul over -0.5*cbT^2), materializes scores to
    SBUF, and produces the per-token running max in the same pass.
  - DVE max_index recovers the argmax index; indirect DMA gathers codebook
    rows for `nearest`.
  - loss via sum ||x||^2 - 2*sum max-score (exact identity), partition-
    reduced with a ones matmul; host sums the 8 per-core partials.

Host side only shards/transposes inputs and concatenates outputs.
"""

import numpy as np

N, K, D = 32768, 8192, 256
NCORES = 8
NSH = N // NCORES  # tokens per core
P = 128

_CACHE = {}


def _build(nsh, k, feats=frozenset({"maxindex", "gather", "loss"})):
    import concourse.bass as bass
    import concourse.tile as tile
    import concourse.mybir as mybir
    from concourse import bacc

    f32 = mybir.dt.float32
    u32 = mybir.dt.uint32
    Alu = mybir.AluOpType
    Act = mybir.ActivationFunctionType
    X = mybir.AxisListType.X
    NEG_INF = -3.0e38

    tt_n = nsh // P            # token tiles
    g_n = k // 2048            # code groups of 2048 (= 4 PSUM banks)
    assert nsh % P == 0 and k % 2048 == 0

    nc = bacc.Bacc("TRN2", target_bir_lowering=False, debug=False)
    xT_d = nc.dram_tensor("xT", [D, nsh], f32, kind="ExternalInput").ap()
    cbT_d = nc.dram_tensor("cbT", [D, k], f32, kind="ExternalInput").ap()
    cb_d = nc.dram_tensor("cb", [k, D], f32, kind="ExternalInput").ap()
    enc_d = nc.dram_tensor("enc", [tt_n, P, 1], u32, kind="ExternalOutput").ap()
    near_d = nc.dram_tensor("nearest", [nsh, D], f32, kind="ExternalOutput").ap()
    loss_d = nc.dram_tensor("losssum", [1, 1], f32, kind="ExternalOutput").ap()

    with tile.TileContext(nc) as tc:
        with (
            tc.tile_pool(name="const", bufs=1) as cpool,
            tc.tile_pool(name="sq", bufs=1) as sqpool,
            tc.tile_pool(name="xt", bufs=4) as xpool,
            tc.tile_pool(name="ps", bufs=2, space="PSUM") as psp,
            tc.tile_pool(name="small", bufs=3) as sm,
            tc.tile_pool(name="near", bufs=4) as npool,
        ):
            # ---- persistent SBUF tensors ----
            cbt0 = cpool.tile([P, k], f32, tag="cbt0")
            cbt1 = cpool.tile([P, k], f32, tag="cbt1")
            bias_rep = cpool.tile([P, k], f32, tag="bias_rep")
            scores = cpool.tile([P, k], f32, tag="scores")
            ones_sq = cpool.tile([P, P], f32, tag="ones_sq")
            ones_col = cpool.tile([P, 1], f32, tag="ones_col")
            zeros8 = cpool.tile([P, 8], f32, tag="zeros8")
            mcol = cpool.tile([P, tt_n], f32, tag="mcol")
            x2col = cpool.tile([P, 2 * tt_n], f32, tag="x2col")

            nc.sync.dma_start(cbt0[:], cbT_d[0:P, :])
            nc.sync.dma_start(cbt1[:], cbT_d[P : 2 * P, :])
            nc.vector.memset(ones_sq[:], 1.0)
            nc.vector.memset(ones_col[:], 1.0)
            nc.vector.memset(zeros8[:], 0.0)

            # ---- bias_rep[p, k] = -0.5 * sum_d cbT[d, k]^2 (replicated) ----
            # sqn = -0.5*cbT^2 staged per 2048-group, then an all-ones
            # [128,128] matmul partition-sums and broadcasts across rows.
            dummy = sm.tile([P, 1], f32, tag="dummy")
            for g in range(g_n):
                gs = g * 2048
                sq0 = sqpool.tile([P, 2048], f32, tag="sq0")
                sq1 = sqpool.tile([P, 2048], f32, tag="sq1")
                if "ttr" in feats:
                    nc.vector.tensor_tensor_reduce(
                        out=sq0[:],
                        in0=cbt0[:, gs : gs + 2048],
                        in1=cbt0[:, gs : gs + 2048],
                        scale=-0.5,
                        scalar=NEG_INF,
                        op0=Alu.mult,
                        op1=Alu.max,
                        accum_out=dummy[:],
                    )
                    nc.vector.tensor_tensor_reduce(
                        out=sq1[:],
                        in0=cbt1[:, gs : gs + 2048],
                        in1=cbt1[:, gs : gs + 2048],
                        scale=-0.5,
                        scalar=NEG_INF,
                        op0=Alu.mult,
                        op1=Alu.max,
                        accum_out=dummy[:],
                    )
                else:
                    nc.vector.tensor_mul(
                        sq0[:], cbt0[:, gs : gs + 2048], cbt0[:, gs : gs + 2048]
                    )
                    nc.vector.tensor_scalar_mul(sq0[:], sq0[:], -0.5)
                    nc.vector.tensor_mul(
                        sq1[:], cbt1[:, gs : gs + 2048], cbt1[:, gs : gs + 2048]
                    )
                    nc.vector.tensor_scalar_mul(sq1[:], sq1[:], -0.5)
                pt = psp.tile([P, 2048], f32, tag="ps")
                for j in range(4):
                    js = j * 512
                    nc.tensor.matmul(
                        pt[:, js : js + 512],
                        ones_sq[:],
                        sq0[:, js : js + 512],
                        start=True,
                        stop=False,
                    )
                    nc.tensor.matmul(
                        pt[:, js : js + 512],
                        ones_sq[:],
                        sq1[:, js : js + 512],
                        start=False,
                        stop=True,
                    )
                nc.vector.tensor_copy(bias_rep[:, gs : gs + 2048], pt[:])

            # ---- main loop over token tiles ----
            for t in range(tt_n):
                xt0 = xpool.tile([P, P], f32, tag="xt0")
                xt1 = xpool.tile([P, P], f32, tag="xt1")
                nc.sync.dma_start(xt0[:], xT_d[0:P, t * P : (t + 1) * P])
                nc.sync.dma_start(xt1[:], xT_d[P : 2 * P, t * P : (t + 1) * P])

                # ||x||^2 partial sums on the Scalar engine (free-axis sum
                # over tokens per d-row; partition-reduced at the end).
                xsq = sm.tile([P, P], f32, tag="xsq")
                nc.scalar.activation(
                    xsq[:], xt0[:], Act.Square, accum_out=x2col[:, 2 * t : 2 * t + 1]
                )
                xsq2 = sm.tile([P, P], f32, tag="xsq2")
                nc.scalar.activation(
                    xsq2[:], xt1[:], Act.Square, accum_out=x2col[:, 2 * t + 1 : 2 * t + 2]
                )

                mprev = None
                mg = sm.tile([P, g_n], f32, tag="mg")
                for g in range(g_n):
                    pt = psp.tile([P, 2048], f32, tag="ps")
                    for j in range(4):
                        cs = g * 2048 + j * 512
                        nc.tensor.matmul(
                            pt[:, j * 512 : (j + 1) * 512],
                            xt0[:],
                            cbt0[:, cs : cs + 512],
                            start=True,
                            stop=False,
                        )
                        nc.tensor.matmul(
                            pt[:, j * 512 : (j + 1) * 512],
                            xt1[:],
                            cbt1[:, cs : cs + 512],
                            start=False,
                            stop=True,
                        )
                    # bias add + materialize + running max, one DVE pass
                    if g == g_n - 1:
                        macc = mcol[:, t : t + 1]
                    else:
                        macc = sm.tile([P, 1], f32, tag=f"mrun{g % 2}")
                    if "ttr" in feats:
                        nc.vector.tensor_tensor_reduce(
                            out=scores[:, g * 2048 : (g + 1) * 2048],
                            in0=pt[:],
                            in1=bias_rep[:, g * 2048 : (g + 1) * 2048],
                            scale=1.0,
                            scalar=NEG_INF if g == 0 else mprev[:],
                            op0=Alu.add,
                            op1=Alu.max,
                            accum_out=macc,
                        )
                    else:
                        nc.vector.tensor_add(
                            scores[:, g * 2048 : (g + 1) * 2048],
                            pt[:],
                            bias_rep[:, g * 2048 : (g + 1) * 2048],
                        )
                        nc.vector.reduce_max(
                            mg[:, g : g + 1],
                            scores[:, g * 2048 : (g + 1) * 2048],
                            axis=X,
                        )
                    mprev = macc
                if "ttr" not in feats:
                    nc.vector.reduce_max(mcol[:, t : t + 1], mg[:], axis=X)

                if "maxindex" in feats:
                    # index recovery
                    needle = sm.tile([P, 8], f32, tag="needle")
                    nc.vector.tensor_scalar_add(
                        needle[:], zeros8[:], mcol[:, t : t + 1]
                    )
                    idx8 = sm.tile([P, 8], u32, tag="idx8")
                    nc.vector.max_index(idx8[:], needle[:], scores[:])
                    nc.sync.dma_start(enc_d[t], idx8[:, 0:1])

                    if "gather" in feats:
                        # nearest = codebook[enc] via indirect row gather
                        nrow = npool.tile([P, D], f32, tag="nrow")
                        nc.gpsimd.indirect_dma_start(
                            out=nrow[:],
                            out_offset=None,
                            in_=cb_d[:, :],
                            in_offset=bass.IndirectOffsetOnAxis(
                                ap=idx8[:, 0:1], axis=0
                            ),
                        )
                        nc.sync.dma_start(near_d[t * P : (t + 1) * P, :], nrow[:])

            # ---- loss partial: sum ||x||^2 - 2 * sum m ----
            if "loss" in feats:
                msum = sm.tile([P, 1], f32, tag="msum")
                nc.vector.reduce_sum(msum[:], mcol[:], axis=X)
                x2s = sm.tile([P, 1], f32, tag="x2s")
                nc.vector.reduce_sum(x2s[:], x2col[:], axis=X)
                part = sm.tile([P, 1], f32, tag="part")
                nc.vector.tensor_scalar(
                    out=part[:], in0=msum[:], scalar1=-2.0, scalar2=None, op0=Alu.mult
                )
                part2 = sm.tile([P, 1], f32, tag="part2")
                nc.vector.tensor_add(part2[:], part[:], x2s[:])
                pl = psp.tile([P, 2048], f32, tag="ps")
                nc.tensor.matmul(
                    pl[0:1, 0:1], part2[:], ones_col[:], start=True, stop=True
                )
                lsb = sm.tile([1, 1], f32, tag="lsb")
                nc.vector.tensor_copy(lsb[:], pl[0:1, 0:1])
                nc.sync.dma_start(loss_d[:, :], lsb[:])

    nc.compile()
    return nc


def _get_nc(nsh, k):
    key = (nsh, k)
    if key not in _CACHE:
        _CACHE[key] = _build(nsh, k)
    return _CACHE[key]


def kernel(x: np.ndarray, codebook: np.ndarray):
    import concourse.bass_utils as bass_utils

    x = np.ascontiguousarray(np.asarray(x, dtype=np.float32))
    codebook = np.ascontiguousarray(np.asarray(codebook, dtype=np.float32))
    assert x.shape == (N, D) and codebook.shape == (K, D)

    nc = _get_nc(NSH, K)
    cbT = np.ascontiguousarray(codebook.T)
    in_maps = []
    for c in range(NCORES):
        xs = x[c * NSH : (c + 1) * NSH]
        in_maps.append(
            {
                "xT": np.ascontiguousarray(xs.T),
                "cbT": cbT,
                "cb": codebook,
            }
        )
    res = bass_utils.run_bass_kernel_spmd(nc, in_maps, core_ids=list(range(NCORES)))

    enc = np.concatenate(
        [res.results[c]["enc"].reshape(-1).astype(np.int32) for c in range(NCORES)]
    )
    nearest = np.concatenate([res.results[c]["nearest"] for c in range(NCORES)])
    loss_sum = np.float64(0.0)
    for c in range(NCORES):
        loss_sum += np.float64(res.results[c]["losssum"][0, 0])
    loss = np.float32(loss_sum / (N * D))
    return enc, loss, loss, nearest
